# revision 50
# baseline (speedup 1.0000x reference)
"""DifferentiableKMeans forward on 8 Trainium2 NeuronCores (Bass/Tile).

Contract: kernel(input_embeddings=[32768,768] f32, centroids=[1024,768] f32)
       -> (clustering_loss: f32 scalar, nearest_centroids: int32 [32768])

Sharding: data-parallel over points (N) across 8 cores; centroids replicated.
Host does layout only (shard + transpose + per-point norms) plus the tiny
cross-core combine: loss = mean_k(S2_k/S1_k) from per-core partial sums
(stable softmin cancels analytically), and an fp64 re-rank of the device's
top-8 argmin candidates for near-tie points (f32r matmul noise).

Device math per core (orientation: points on partitions, centroids on free):
  PSUM P[n,k] = 2048*x.c - 1024*x2[n] - 1024*c2[k] = -1024*d2[n,k]
    - x.c via f32r (rounded-fp32) matmuls, full PE rate
    - x2/c2 folded as 3 extra contraction rows (c2 split hi+lo for accuracy)
  d = sqrt(-P/1024)        ScalarE
  E = exp(-d)   -> bf16    ScalarE
  F = d*E       -> bf16    DVE
  top-8 min d2 + indices:  DVE max/max_index on P (max of -d2)
  S1[k] = sum_n E, S2[k] = sum_n F: ones-vector matmuls accumulated in PSUM
"""
import os
import numpy as np

N_CORES = 8
N, D, K = 32768, 768, 1024
NPTS = N // N_CORES
P = 128
NCHUNK = D // P
GRP = 4
NTILES = NPTS // P
NGROUPS = NTILES // GRP

_COMPILED = {}


def _round_f32r(a):
    """Round fp32 to the PE's fp32r grid: RNE dropping the low 12 mantissa
    bits (verified bit-exact against the on-device cast)."""
    bits = np.ascontiguousarray(a, np.float32).view(np.uint32)
    r = bits + 0x7FF + ((bits >> 12) & 1)
    return (r & np.uint32(0xFFFFF000)).view(np.float32)


def _setup_act_root():
    """Point walrus at an act-table root whose only set is
    natural_log_exp_and_others (exp+ln+square+copy), so every ScalarE
    function resolves to one table set and no per-tile reloads happen."""
    import json, shutil, neuronxcc
    dst = "/tmp/dkm_act_root"
    marker = os.path.join(dst, "act_info.json")
    if not os.path.exists(marker):
        src = os.path.join(os.path.dirname(neuronxcc.__file__),
                           "pwp", "pwp_bin_trainium")
        shutil.copytree(src, dst, dirs_exist_ok=True)
        with open(os.path.join(src, "act_info.json")) as f:
            info = json.load(f)
        keep = [s for s in info["act_func_sets"]
                if s["name"] == "natural_log_exp_and_others"]
        assert keep, "natural_log_exp_and_others set missing from act_info"
        info["act_func_sets"] = keep
        with open(marker, "w") as f:
            json.dump(info, f)
    os.environ["BASS_ACT_ROOT_JSON_PATH"] = marker

    # keep the bass-side ATL placement consistent with the 1-set root
    import concourse.hw_specs as hw_specs
    import concourse.bacc as bacc_mod
    orig = hw_specs.get_activation_tables
    if not getattr(hw_specs, "_dkm_patched", False):
        def only_nle(arch, _orig=orig):
            t = _orig(arch)
            return {"natural_log_exp_and_others": t["natural_log_exp_and_others"]}
        hw_specs.get_activation_tables = only_nle
        hw_specs._dkm_patched = True
        if getattr(bacc_mod, "get_activation_tables", None) is not None:
            bacc_mod.get_activation_tables = only_nle


def _build():
    import concourse.bacc as bacc
    import concourse.mybir as mybir
    from concourse.alu_op_type import AluOpType
    from concourse.tile import TileContext

    F32 = mybir.dt.float32
    F32R = mybir.dt.float32r
    BF16 = mybir.dt.bfloat16
    U32 = mybir.dt.uint32
    AF = mybir.ActivationFunctionType

    nc = bacc.Bacc("TRN2", target_bir_lowering=False, debug=False,
                   num_devices=N_CORES)

    xt_d = nc.declare_dram_parameter("xt", [D, NPTS], F32R, isOutput=False)
    x2_d = nc.declare_dram_parameter("x2", [P, NTILES], F32, isOutput=False)
    ct_d = nc.declare_dram_parameter("ct", [D, K], F32R, isOutput=False)
    exr_d = nc.declare_dram_parameter("exr", [2, K], F32R, isOutput=False)
    idx_d = nc.declare_dram_parameter("idx", [NPTS, 8], U32, isOutput=True)
    val_d = nc.declare_dram_parameter("val", [NPTS, 8], F32, isOutput=True)
    s1_d = nc.declare_dram_parameter("s1", [1, K], F32, isOutput=True)
    s2_d = nc.declare_dram_parameter("s2", [1, K], F32, isOutput=True)

    with TileContext(nc) as tc:
        with tc.tile_pool(name="const", bufs=1) as const, \
             tc.tile_pool(name="xtp", bufs=3) as xtp, \
             tc.tile_pool(name="work", bufs=3) as work, \
             tc.tile_pool(name="stage", bufs=1) as stage, \
             tc.tile_pool(name="ppool", bufs=3, space="PSUM") as ppool, \
             tc.tile_pool(name="accp", bufs=1, space="PSUM") as accp:

            # ---------- setup ----------
            xt_view = xt_d[:].rearrange("(c p) n -> p c n", p=P)

            ct_view = ct_d[:].rearrange("(c p) k -> p c k", p=P)

            def load_group(g, split=False):
                xt_g = xtp.tile([P, NCHUNK, GRP * P], F32R, name="xt_g")
                sl = slice(g * GRP * P, (g + 1) * GRP * P)
                if split:
                    for c in range(NCHUNK):
                        nc.sync.dma_start(out=xt_g[:, c, :], in_=xt_view[:, c, sl])
                else:
                    nc.sync.dma_start(out=xt_g[:], in_=xt_view[:, :, sl])
                return xt_g

            # head: interleave CT chunks with the first X group chunk-by-chunk
            # so tile 0's accumulation can start as soon as chunk 0 lands
            ex_r = const.tile([2, K], F32R, name="ex_r")
            nc.sync.dma_start(out=ex_r[:], in_=exr_d[:])
            x2g = const.tile([P, NTILES], F32, name="x2g")
            nc.sync.dma_start(out=x2g[:], in_=x2_d[:])
            ct_r = const.tile([P, NCHUNK, K], F32R, name="ct_r")
            xt_g0 = xtp.tile([P, NCHUNK, GRP * P], F32R, name="xt_g")
            for c in range(NCHUNK):
                nc.sync.dma_start(out=ct_r[:, c, :], in_=ct_view[:, c, :])
                nc.sync.dma_start(out=xt_g0[:, c, :], in_=xt_view[:, c, 0:GRP * P])
            pending = {0: xt_g0, 1: load_group(1, split=True)}

            ones_r = const.tile([2, P], F32R, name="ones_r")
            nc.gpsimd.memset(ones_r[:].bitcast(F32), 1.0)
            ones_bf = const.tile([P, 1], BF16, name="ones_bf")
            nc.gpsimd.memset(ones_bf[:], 1.0)

            idx_st = stage.tile([P, NTILES, 8], U32, name="idx_st")
            val_st = stage.tile([P, NTILES, 8], F32, name="val_st")
            # S1 on partition 0, S2 on partition 32 of one PSUM tile (2 banks)
            acc = accp.tile([P, K], F32, name="acc")

            ef_hist = {}
            de_hist = {}

            def emit_f(t):
                d_p, e_p = de_hist.pop(t)
                f_bf = work.tile([P, K], BF16, name="f_bf", bufs=4)
                nc.vector.tensor_tensor(f_bf[:], d_p[:], e_p[:], AluOpType.mult)
                ef_hist[t] = (e_p, f_bf)

            def emit_colsums(t):
                # all four sums in distinct PE col-groups -> fully concurrent
                # rows of acc: 0 = S1 (k 0:512), 32 = S2 (k 0:512),
                #              64 = S1 (k 512:1024), 96 = S2 (k 512:1024)
                e_bf, f_bf = ef_hist.pop(t)
                for h in range(2):
                    nc.tensor.matmul(acc[64 * h:64 * h + 1, h * 512:(h + 1) * 512],
                                     ones_bf[:], e_bf[:, h * 512:(h + 1) * 512],
                                     start=(t == 0), stop=(t == NTILES - 1),
                                     skip_group_check=True,
                                     tile_position=(0, 64 * h))
                    nc.tensor.matmul(acc[64 * h + 32:64 * h + 33, h * 512:(h + 1) * 512],
                                     ones_bf[:], f_bf[:, h * 512:(h + 1) * 512],
                                     start=(t == 0), stop=(t == NTILES - 1),
                                     skip_group_check=True,
                                     tile_position=(0, 64 * h + 32))

            for g in range(NGROUPS):
                if g + 2 < NGROUPS and g + 2 not in pending:
                    pending[g + 2] = load_group(g + 2)
                xt_g = pending.pop(g)

                for j in range(GRP):
                    t = g * GRP + j
                    pt = ppool.tile([P, K], F32, name="pt")
                    # bank-contiguous order: all seven matmuls of PSUM bank h
                    # before switching banks (bank cycling causes PE
                    # micro-idles); extras open each bank group - tiny
                    # ldweights + always-ready operands absorb the tile-start
                    # PSUM wait
                    for h in range(2):
                        sl = slice(h * 512, (h + 1) * 512)
                        nc.tensor.matmul(pt[:, sl], ones_r[:], ex_r[:, sl],
                                         start=True, stop=False)
                        for c in range(NCHUNK):
                            nc.tensor.matmul(pt[:, sl],
                                             xt_g[:, c, j * P:(j + 1) * P],
                                             ct_r[:, c, sl],
                                             start=False, stop=(c == NCHUNK - 1))

                    # d = sqrt(d2), d2 = -P/1024 + x2 (x2 via the act bias),
                    # computed as exp(0.5*ln(.)) so the whole kernel stays
                    # inside one ACT table set (ln+exp+square)
                    u_t = work.tile([P, K], F32, name="u_t", bufs=4)
                    nc.scalar.activation(u_t[:], pt[:], AF.Ln, scale=-1.0 / 1024.0,
                                         bias=x2g[:, t:t + 1])
                    d_t = work.tile([P, K], F32, name="d_t", bufs=4)
                    nc.scalar.activation(d_t[:], u_t[:], AF.Exp, scale=0.5)
                    # argmin ops first: they are the last PSUM readers, so
                    # running them early releases P banks for tile t+3
                    nc.vector.max(val_st[:, t, :], pt[:])
                    nc.vector.max_index(idx_st[:, t, :], val_st[:, t, :], pt[:])

                    e_bf = work.tile([P, K], BF16, name="e_bf", bufs=4)
                    nc.scalar.activation(e_bf[:], d_t[:], AF.Exp, scale=-1.0)
                    de_hist[t] = (d_t, e_bf)

                    # F lags one tile: when the DVE reaches it, E is long
                    # ready, so it never head-of-line-blocks the next tile's
                    # PSUM-releasing max/max_index
                    if t > 0:
                        emit_f(t - 1)
                    # colsums lag two tiles for the same reason on the PE
                    if t > 1:
                        emit_colsums(t - 2)

                # stream this group's idx/val out while compute continues
                ts0, ts1 = g * GRP, (g + 1) * GRP
                nc.sync.dma_start(
                    out=idx_d[:].rearrange("(t p) j -> p t j", p=P)[:, ts0:ts1, :],
                    in_=idx_st[:, ts0:ts1, :])
                nc.sync.dma_start(
                    out=val_d[:].rearrange("(t p) j -> p t j", p=P)[:, ts0:ts1, :],
                    in_=val_st[:, ts0:ts1, :])
            emit_f(NTILES - 1)
            emit_colsums(NTILES - 2)
            emit_colsums(NTILES - 1)

            # ---------- drain ----------
            # drain: halves live at partition 0/32 (k 0:512) and 64/96 (k 512:)
            s1sb = stage.tile([1, K], F32, name="s1sb")
            s2sb = stage.tile([1, K], F32, name="s2sb")
            nc.scalar.copy(out=s1sb[:, 0:512], in_=acc[0:1, 0:512])
            nc.scalar.copy(out=s1sb[:, 512:1024], in_=acc[64:65, 512:1024])
            nc.vector.tensor_copy(s2sb[:, 0:512], acc[32:33, 0:512])
            nc.vector.tensor_copy(s2sb[:, 512:1024], acc[96:97, 512:1024])
            nc.sync.dma_start(out=s1_d[:], in_=s1sb[:])
            nc.sync.dma_start(out=s2_d[:], in_=s2sb[:])

    nc.compile()
    return nc


def _get_compiled():
    if "nc" not in _COMPILED:
        _setup_act_root()
        _COMPILED["nc"] = _build()
    return _COMPILED["nc"]


def kernel(input_embeddings, centroids):
    from concourse.bass_utils import run_bass_kernel_spmd

    X = np.ascontiguousarray(np.asarray(input_embeddings, dtype=np.float32))
    C = np.ascontiguousarray(np.asarray(centroids, dtype=np.float32))
    assert X.shape == (N, D) and C.shape == (K, D)

    nc = _get_compiled()

    # layout/encoding prep: transpose + scale-by-2^11 + round to the PE's
    # fp32r grid.  c2 is computed from the *rounded* centroids so the device
    # distance geometry is self-consistent.
    XT_r = _round_f32r(X.T)
    CT_r = _round_f32r(2048.0 * C.T)
    c2 = np.einsum('dk,dk->k', CT_r.astype(np.float64), CT_r.astype(np.float64)) \
        * (2.0 ** -22)
    t = (-1024.0 * c2)
    hi = _round_f32r(t.astype(np.float32))
    lo = _round_f32r((t - hi.astype(np.float64)).astype(np.float32))
    EXR = np.ascontiguousarray(np.stack([hi, lo]).astype(np.float32))
    in_maps = []
    for i in range(N_CORES):
        Xs = X[i * NPTS:(i + 1) * NPTS]
        x2 = np.einsum('nd,nd->n', Xs, Xs, dtype=np.float64).astype(np.float32)
        in_maps.append({
            "xt": np.ascontiguousarray(XT_r[:, i * NPTS:(i + 1) * NPTS]),
            "x2": np.ascontiguousarray(x2.reshape(NTILES, P).T),
            "ct": CT_r,
            "exr": EXR,
        })

    try:
        res = run_bass_kernel_spmd(nc, in_maps, list(range(N_CORES)))
    except Exception:
        # one retry for transient device errors
        res = run_bass_kernel_spmd(nc, in_maps, list(range(N_CORES)))

    # ---- host combine (fp64) ----
    S1 = np.zeros(K, np.float64)
    S2 = np.zeros(K, np.float64)
    nearest = np.empty(N, np.int64)
    idx_all = np.empty((N, 8), np.int64)
    val_all = np.empty((N, 8), np.float64)
    for i in range(N_CORES):
        r = res.results[i]
        S1 += r["s1"][0].astype(np.float64)
        S2 += r["s2"][0].astype(np.float64)
        idx_all[i * NPTS:(i + 1) * NPTS] = r["idx"].astype(np.int64)
        val_all[i * NPTS:(i + 1) * NPTS] = r["val"].astype(np.float64)

    loss = np.float32((S2 / S1).mean())

    nearest = idx_all[:, 0].copy()
    # near-tie fixup: device distances carry ~0.01 d2 noise (f32r matmuls);
    # re-rank the top-8 candidates in fp64 where the top-2 gap is small.
    gap = (val_all[:, 0] - val_all[:, 1]) / 1024.0   # d2 gap, top1 vs top2
    flagged = np.nonzero(gap < 0.25)[0]
    if flagged.size:
        X64 = X.astype(np.float64)
        C64 = C.astype(np.float64)
        c2_64 = np.einsum('kd,kd->k', C64, C64)
        for n in flagged:
            cand = idx_all[n]
            d2c = (X64[n] @ X64[n]) + c2_64[cand] - 2.0 * (C64[cand] @ X64[n])
            nearest[n] = cand[np.argmin(d2c)]

    return loss, nearest.astype(np.int32)


# revision 52
# speedup vs baseline: 1.1280x; 1.1280x over previous
"""DifferentiableKMeans forward on 8 Trainium2 NeuronCores (Bass/Tile).

Contract: kernel(input_embeddings=[32768,768] f32, centroids=[1024,768] f32)
       -> (clustering_loss: f32 scalar, nearest_centroids: int32 [32768])

Sharding: data-parallel over points (N) across 8 cores; centroids replicated.
Host does layout only (shard + transpose + per-point norms) plus the tiny
cross-core combine: loss = mean_k(S2_k/S1_k) from per-core partial sums
(stable softmin cancels analytically), and an fp64 re-rank of the device's
top-8 argmin candidates for near-tie points (f32r matmul noise).

Device math per core (orientation: points on partitions, centroids on free):
  PSUM P[n,k] = 2048*x.c - 1024*x2[n] - 1024*c2[k] = -1024*d2[n,k]
    - x.c via f32r (rounded-fp32) matmuls, full PE rate
    - x2/c2 folded as 3 extra contraction rows (c2 split hi+lo for accuracy)
  d = sqrt(-P/1024)        ScalarE
  E = exp(-d)   -> bf16    ScalarE
  F = d*E       -> bf16    DVE
  top-8 min d2 + indices:  DVE max/max_index on P (max of -d2)
  S1[k] = sum_n E, S2[k] = sum_n F: ones-vector matmuls accumulated in PSUM
"""
import os
import numpy as np

N_CORES = 8
N, D, K = 32768, 768, 1024
NPTS = N // N_CORES
P = 128
NCHUNK = D // P
GRP = 4
NTILES = NPTS // P
NGROUPS = NTILES // GRP

_COMPILED = {}


def _round_f32r(a):
    """Round fp32 to the PE's fp32r grid: RNE dropping the low 12 mantissa
    bits (verified bit-exact against the on-device cast)."""
    bits = np.ascontiguousarray(a, np.float32).view(np.uint32)
    r = bits + 0x7FF + ((bits >> 12) & 1)
    return (r & np.uint32(0xFFFFF000)).view(np.float32)


def _setup_act_root():
    """Point walrus at an act-table root whose only set is
    natural_log_exp_and_others (exp+ln+square+copy), so every ScalarE
    function resolves to one table set and no per-tile reloads happen."""
    import json, shutil, neuronxcc
    dst = "/tmp/dkm_act_root"
    marker = os.path.join(dst, "act_info.json")
    if not os.path.exists(marker):
        src = os.path.join(os.path.dirname(neuronxcc.__file__),
                           "pwp", "pwp_bin_trainium")
        shutil.copytree(src, dst, dirs_exist_ok=True)
        with open(os.path.join(src, "act_info.json")) as f:
            info = json.load(f)
        keep = [s for s in info["act_func_sets"]
                if s["name"] == "natural_log_exp_and_others"]
        assert keep, "natural_log_exp_and_others set missing from act_info"
        info["act_func_sets"] = keep
        with open(marker, "w") as f:
            json.dump(info, f)
    os.environ["BASS_ACT_ROOT_JSON_PATH"] = marker

    # keep the bass-side ATL placement consistent with the 1-set root
    import concourse.hw_specs as hw_specs
    import concourse.bacc as bacc_mod
    orig = hw_specs.get_activation_tables
    if not getattr(hw_specs, "_dkm_patched", False):
        def only_nle(arch, _orig=orig):
            t = _orig(arch)
            return {"natural_log_exp_and_others": t["natural_log_exp_and_others"]}
        hw_specs.get_activation_tables = only_nle
        hw_specs._dkm_patched = True
        if getattr(bacc_mod, "get_activation_tables", None) is not None:
            bacc_mod.get_activation_tables = only_nle


def _build():
    import concourse.bacc as bacc
    import concourse.mybir as mybir
    from concourse.alu_op_type import AluOpType
    from concourse.tile import TileContext

    F32 = mybir.dt.float32
    F32R = mybir.dt.float32r
    BF16 = mybir.dt.bfloat16
    U32 = mybir.dt.uint32
    AF = mybir.ActivationFunctionType

    nc = bacc.Bacc("TRN2", target_bir_lowering=False, debug=False,
                   num_devices=N_CORES)

    xt_d = nc.declare_dram_parameter("xt", [D, NPTS], F32R, isOutput=False)
    x2_d = nc.declare_dram_parameter("x2", [P, NTILES], F32, isOutput=False)
    ct_d = nc.declare_dram_parameter("ct", [D, K], F32R, isOutput=False)
    exr_d = nc.declare_dram_parameter("exr", [2, K], F32R, isOutput=False)
    idx_d = nc.declare_dram_parameter("idx", [NPTS, 8], U32, isOutput=True)
    val_d = nc.declare_dram_parameter("val", [NPTS, 8], F32, isOutput=True)
    s1_d = nc.declare_dram_parameter("s1", [1, K], F32, isOutput=True)
    s2_d = nc.declare_dram_parameter("s2", [1, K], F32, isOutput=True)

    with TileContext(nc) as tc:
        with tc.tile_pool(name="const", bufs=1) as const, \
             tc.tile_pool(name="xtp", bufs=3) as xtp, \
             tc.tile_pool(name="work", bufs=3) as work, \
             tc.tile_pool(name="stage", bufs=1) as stage, \
             tc.tile_pool(name="ppool", bufs=3, space="PSUM") as ppool, \
             tc.tile_pool(name="accp", bufs=1, space="PSUM") as accp:

            # ---------- setup ----------
            xt_view = xt_d[:].rearrange("(c p) n -> p c n", p=P)

            ct_view = ct_d[:].rearrange("(c p) k -> p c k", p=P)

            def load_group(g, split=False):
                xt_g = xtp.tile([P, NCHUNK, GRP * P], F32R, name="xt_g")
                sl = slice(g * GRP * P, (g + 1) * GRP * P)
                if split:
                    for c in range(NCHUNK):
                        nc.sync.dma_start(out=xt_g[:, c, :], in_=xt_view[:, c, sl])
                else:
                    nc.sync.dma_start(out=xt_g[:], in_=xt_view[:, :, sl])
                return xt_g

            # head: interleave CT chunks with the first X group chunk-by-chunk
            # so tile 0's accumulation can start as soon as chunk 0 lands
            ex_r = const.tile([2, K], F32R, name="ex_r")
            nc.sync.dma_start(out=ex_r[:], in_=exr_d[:])
            x2g = const.tile([P, NTILES], F32, name="x2g")
            nc.sync.dma_start(out=x2g[:], in_=x2_d[:])
            ct_r = const.tile([P, NCHUNK, K], F32R, name="ct_r")
            xt_g0 = xtp.tile([P, NCHUNK, GRP * P], F32R, name="xt_g")
            for c in range(NCHUNK):
                nc.sync.dma_start(out=ct_r[:, c, :], in_=ct_view[:, c, :])
                nc.sync.dma_start(out=xt_g0[:, c, :], in_=xt_view[:, c, 0:GRP * P])
            pending = {0: xt_g0, 1: load_group(1, split=True)}

            ones_r = const.tile([2, P], F32R, name="ones_r")
            nc.gpsimd.memset(ones_r[:].bitcast(F32), 1.0)
            ones_bf = const.tile([P, 1], BF16, name="ones_bf")
            nc.gpsimd.memset(ones_bf[:], 1.0)

            idx_st = stage.tile([P, NTILES, 8], U32, name="idx_st")
            val_st = stage.tile([P, NTILES, 8], F32, name="val_st")
            # S1 on partition 0, S2 on partition 32 of one PSUM tile (2 banks)
            acc = accp.tile([P, K], F32, name="acc")

            ef_hist = {}
            de_hist = {}

            def emit_f(t):
                d_p, e_p = de_hist.pop(t)
                f_bf = work.tile([P, K], BF16, name="f_bf", bufs=6)
                nc.vector.tensor_tensor(f_bf[:], d_p[:], e_p[:], AluOpType.mult)
                ef_hist[t] = (e_p, f_bf)

            def emit_colsums(t):
                # all four sums in distinct PE col-groups -> fully concurrent
                # rows of acc: 0 = S1 (k 0:512), 32 = S2 (k 0:512),
                #              64 = S1 (k 512:1024), 96 = S2 (k 512:1024)
                e_bf, f_bf = ef_hist.pop(t)
                for h in range(2):
                    nc.tensor.matmul(acc[64 * h:64 * h + 1, h * 512:(h + 1) * 512],
                                     ones_bf[:], e_bf[:, h * 512:(h + 1) * 512],
                                     start=(t == 0), stop=(t == NTILES - 1),
                                     skip_group_check=True,
                                     tile_position=(0, 64 * h))
                    nc.tensor.matmul(acc[64 * h + 32:64 * h + 33, h * 512:(h + 1) * 512],
                                     ones_bf[:], f_bf[:, h * 512:(h + 1) * 512],
                                     start=(t == 0), stop=(t == NTILES - 1),
                                     skip_group_check=True,
                                     tile_position=(0, 64 * h + 32))

            for g in range(NGROUPS):
                if g + 2 < NGROUPS and g + 2 not in pending:
                    pending[g + 2] = load_group(g + 2)
                xt_g = pending.pop(g)

                for j in range(GRP):
                    t = g * GRP + j
                    pt = ppool.tile([P, K], F32, name="pt")
                    # extras first: tiny ldweights + always-ready operands, so
                    # the tile-start PSUM wait lands here and the first main's
                    # weight load hides under their streams
                    for h in range(2):
                        nc.tensor.matmul(pt[:, h * 512:(h + 1) * 512],
                                         ones_r[:],
                                         ex_r[:, h * 512:(h + 1) * 512],
                                         start=True, stop=False)
                    for c in range(NCHUNK):
                        for h in range(2):
                            nc.tensor.matmul(pt[:, h * 512:(h + 1) * 512],
                                             xt_g[:, c, j * P:(j + 1) * P],
                                             ct_r[:, c, h * 512:(h + 1) * 512],
                                             start=False, stop=(c == NCHUNK - 1))

                    # d = sqrt(d2), d2 = -P/1024 + x2 (x2 via the act bias),
                    # computed as exp(0.5*ln(.)) so the whole kernel stays
                    # inside one ACT table set (ln+exp+square)
                    u_t = work.tile([P, K], F32, name="u_t", bufs=4)
                    nc.scalar.activation(u_t[:], pt[:], AF.Ln, scale=-1.0 / 1024.0,
                                         bias=x2g[:, t:t + 1])
                    d_t = work.tile([P, K], F32, name="d_t", bufs=6)
                    nc.scalar.activation(d_t[:], u_t[:], AF.Exp, scale=0.5)
                    # argmin ops first: they are the last PSUM readers, so
                    # running them early releases P banks for tile t+3
                    nc.vector.max(val_st[:, t, :], pt[:])
                    nc.vector.max_index(idx_st[:, t, :], val_st[:, t, :], pt[:])

                    e_bf = work.tile([P, K], BF16, name="e_bf", bufs=6)
                    nc.scalar.activation(e_bf[:], d_t[:], AF.Exp, scale=-1.0)
                    de_hist[t] = (d_t, e_bf)

                    # F lags one tile: when the DVE reaches it, E is long
                    # ready, so it never head-of-line-blocks the next tile's
                    # PSUM-releasing max/max_index
                    if t > 0:
                        emit_f(t - 1)
                    # colsums lag two tiles for the same reason on the PE
                    if t > 1:
                        emit_colsums(t - 2)

                # stream this group's idx/val out while compute continues
                ts0, ts1 = g * GRP, (g + 1) * GRP
                nc.sync.dma_start(
                    out=idx_d[:].rearrange("(t p) j -> p t j", p=P)[:, ts0:ts1, :],
                    in_=idx_st[:, ts0:ts1, :])
                nc.sync.dma_start(
                    out=val_d[:].rearrange("(t p) j -> p t j", p=P)[:, ts0:ts1, :],
                    in_=val_st[:, ts0:ts1, :])
            emit_f(NTILES - 1)
            emit_colsums(NTILES - 2)
            emit_colsums(NTILES - 1)

            # ---------- drain ----------
            # drain: halves live at partition 0/32 (k 0:512) and 64/96 (k 512:)
            s1sb = stage.tile([1, K], F32, name="s1sb")
            s2sb = stage.tile([1, K], F32, name="s2sb")
            nc.scalar.copy(out=s1sb[:, 0:512], in_=acc[0:1, 0:512])
            nc.scalar.copy(out=s1sb[:, 512:1024], in_=acc[64:65, 512:1024])
            nc.vector.tensor_copy(s2sb[:, 0:512], acc[32:33, 0:512])
            nc.vector.tensor_copy(s2sb[:, 512:1024], acc[96:97, 512:1024])
            nc.sync.dma_start(out=s1_d[:], in_=s1sb[:])
            nc.sync.dma_start(out=s2_d[:], in_=s2sb[:])

    nc.compile()
    return nc


def _get_compiled():
    if "nc" not in _COMPILED:
        _setup_act_root()
        _COMPILED["nc"] = _build()
    return _COMPILED["nc"]


def kernel(input_embeddings, centroids):
    from concourse.bass_utils import run_bass_kernel_spmd

    X = np.ascontiguousarray(np.asarray(input_embeddings, dtype=np.float32))
    C = np.ascontiguousarray(np.asarray(centroids, dtype=np.float32))
    assert X.shape == (N, D) and C.shape == (K, D)

    nc = _get_compiled()

    # layout/encoding prep: transpose + scale-by-2^11 + round to the PE's
    # fp32r grid.  c2 is computed from the *rounded* centroids so the device
    # distance geometry is self-consistent.
    XT_r = _round_f32r(X.T)
    CT_r = _round_f32r(2048.0 * C.T)
    c2 = np.einsum('dk,dk->k', CT_r.astype(np.float64), CT_r.astype(np.float64)) \
        * (2.0 ** -22)
    t = (-1024.0 * c2)
    hi = _round_f32r(t.astype(np.float32))
    lo = _round_f32r((t - hi.astype(np.float64)).astype(np.float32))
    EXR = np.ascontiguousarray(np.stack([hi, lo]).astype(np.float32))
    in_maps = []
    for i in range(N_CORES):
        Xs = X[i * NPTS:(i + 1) * NPTS]
        x2 = np.einsum('nd,nd->n', Xs, Xs, dtype=np.float64).astype(np.float32)
        in_maps.append({
            "xt": np.ascontiguousarray(XT_r[:, i * NPTS:(i + 1) * NPTS]),
            "x2": np.ascontiguousarray(x2.reshape(NTILES, P).T),
            "ct": CT_r,
            "exr": EXR,
        })

    try:
        res = run_bass_kernel_spmd(nc, in_maps, list(range(N_CORES)))
    except Exception:
        # one retry for transient device errors
        res = run_bass_kernel_spmd(nc, in_maps, list(range(N_CORES)))

    # ---- host combine (fp64) ----
    S1 = np.zeros(K, np.float64)
    S2 = np.zeros(K, np.float64)
    nearest = np.empty(N, np.int64)
    idx_all = np.empty((N, 8), np.int64)
    val_all = np.empty((N, 8), np.float64)
    for i in range(N_CORES):
        r = res.results[i]
        S1 += r["s1"][0].astype(np.float64)
        S2 += r["s2"][0].astype(np.float64)
        idx_all[i * NPTS:(i + 1) * NPTS] = r["idx"].astype(np.int64)
        val_all[i * NPTS:(i + 1) * NPTS] = r["val"].astype(np.float64)

    loss = np.float32((S2 / S1).mean())

    nearest = idx_all[:, 0].copy()
    # near-tie fixup: device distances carry ~0.01 d2 noise (f32r matmuls);
    # re-rank the top-8 candidates in fp64 where the top-2 gap is small.
    gap = (val_all[:, 0] - val_all[:, 1]) / 1024.0   # d2 gap, top1 vs top2
    flagged = np.nonzero(gap < 0.25)[0]
    if flagged.size:
        X64 = X.astype(np.float64)
        C64 = C.astype(np.float64)
        c2_64 = np.einsum('kd,kd->k', C64, C64)
        for n in flagged:
            cand = idx_all[n]
            d2c = (X64[n] @ X64[n]) + c2_64[cand] - 2.0 * (C64[cand] @ X64[n])
            nearest[n] = cand[np.argmin(d2c)]

    return loss, nearest.astype(np.int32)
